# revision 22
# baseline (speedup 1.0000x reference)
"""Bass/Trainium2 kernel for nn_HMEClassification (hierarchical mixture-of-experts).

Strategy: pure data parallel across 8 cores (batch sharded). Per core:
  xT [128d, 16384b] streamed in 512-wide b-tiles (bf16).
  L1 (7 units: 3 gates + 4 experts): weight-stationary bf16 matmuls
      lhsT=W1 block [128d,128h], rhs=xT tile [128d,512b] -> PSUM [128h,512b],
      fp32 accumulate. Blocks evacuated in PAIRS ([128,1024] spanning two
      PSUM banks) with fused relu on ScalarE/VectorE - halves instruction
      count vs per-block evacuation (biases are zero per problem spec;
      general per-block fallback kept for nonzero biases).
  L2 experts: col-tiled pairs, lhsT=eW2 chunk [128h,64c], K-accumulated over
      4 h-chunks -> PSUM [128(=2x64c), 512b] logits^T.
  Gates: softmax over 2 == sigmoid(z0-z1) with difference weights. All four
      gate matmul positions live in ONE PSUM bank at four distinct PE column
      groups so they run concurrently on the array:
        rows {0,1}=+d_root,+d_root @(0,0)    rows {32,33}=-d_root,-d_root @(0,32)
        rows {64,65}=d_A,-d_A @(0,64)        rows {96,97}=d_B,-d_B @(0,96)
      sigmoid -> TT rows{0..33}=(rA,rA|rB,rB), rows{64..97}=(gA0,gA1|gB0,gB1).
      A small SBUF->SBUF DMA replicates rows 0:34 to rows 64:98 so the
      root*gate product is lane-aligned at rows {64,65},{96,97}.
  Softmax over classes: exp on ScalarE (logits are O(1), no max needed),
      partition sums via ones-select matmul at col groups 2,3 (S at rows
      {64,65},{96,97}), C = P/S, DMA partition-broadcast of C rows to
      64-partition blocks, prod = exp * C in one [128,1024] multiply,
      final 4-expert sum via stacked-identity matmul.
  Output out^T [64, 16384] fp32 per core; host transposes/concats.
"""

import ml_dtypes
import numpy as np

import concourse.bass as bass
import concourse.mybir as mybir
import concourse.tile as tile
from concourse import bacc
from concourse.bass_utils import run_bass_kernel_spmd

B, D, H, C = 131072, 128, 512, 64
NCORES = 8
BC = B // NCORES        # 16384 rows per core
TB = 512                # b-tile width
KH = H // 128           # 4 h-chunks of 128

F32 = mybir.dt.float32
BF16 = mybir.dt.bfloat16

# ---- bf16 consts layout (columns in [128, NB] bf16 tensor) ----
W1_OFF = 0                       # 7 units * 512 = 3584
W2_OFF = W1_OFF + 7 * H          # 16 blocks (k*4+e) * 64 = 1024
GP_OFF = W2_OFF + 16 * 64        # 4 chunks * 8 = 32
OS_OFF = GP_OFF + 32             # 2 cols (ones select)
ID_OFF = OS_OFF + 2              # 64 cols (stacked identity)
NB = ID_OFF + 64
# ---- fp32 consts layout ----
B1_OFF = 0                       # 28 cols (u*4+hb)
EB_OFF = B1_OFF + 28             # 2 cols
GB_OFF = EB_OFF + 2              # 1 col (packed sigmoid biases, rows
                                 #        {0,1},{32,33},{64,65},{96,97})
NF = GB_OFF + 1

# L1 block order: (u, hb) pairs sharing one [128,1024] PSUM tile + evac op
L1_BLOCKS = [(u, hb) for u in range(7) for hb in range(KH)]
N_PAIRS = len(L1_BLOCKS) // 2    # 14
# evac engine per pair: ScalarE ~427ns/512col vs DVE ~533ns; balance with
# each engine's fixed per-tile work (exp+sigmoid on ACT; coeff chain on DVE)
PAIR_ON_SCALAR = [True, False, True, False, True, False, True, False,
                  True, False, True, False, True, False]


def _build_consts(gW1, gb1, gW2, gb2, eW1, eb1, eW2, eb2):
    cb = np.zeros((128, NB), dtype=np.float32)
    for u in range(3):
        cb[:, W1_OFF + u * H: W1_OFF + (u + 1) * H] = gW1[u]
    for e in range(4):
        cb[:, W1_OFF + (3 + e) * H: W1_OFF + (4 + e) * H] = eW1[e]
    for k in range(KH):
        for e in range(4):
            cb[:, W2_OFF + (k * 4 + e) * 64: W2_OFF + (k * 4 + e + 1) * 64] = \
                eW2[e, k * 128:(k + 1) * 128, :]
    v = gW2[:, :, 0] - gW2[:, :, 1]          # [3, 512]
    for k in range(KH):
        sl = slice(k * 128, (k + 1) * 128)
        blk = np.stack([v[0, sl], v[0, sl], -v[0, sl], -v[0, sl],
                        v[1, sl], -v[1, sl], v[2, sl], -v[2, sl]], axis=1)
        cb[:, GP_OFF + k * 8: GP_OFF + (k + 1) * 8] = blk
    cb[:64, OS_OFF + 0] = 1.0
    cb[64:, OS_OFF + 1] = 1.0
    p = np.arange(128)
    cb[:, ID_OFF: ID_OFF + 64] = (p[:, None] % 64 == np.arange(64)[None, :])

    cf = np.zeros((128, NF), dtype=np.float32)
    b1 = np.concatenate([gb1, eb1], axis=0)  # [7, 512]
    for u in range(7):
        for hb in range(KH):
            cf[:, B1_OFF + u * 4 + hb] = b1[u, hb * 128:(hb + 1) * 128]
    cf[:64, EB_OFF + 0] = eb2[0]
    cf[64:, EB_OFF + 0] = eb2[1]
    cf[:64, EB_OFF + 1] = eb2[2]
    cf[64:, EB_OFF + 1] = eb2[3]
    # negated packed gate biases: used as exp(-z - db) = exp(scale*z + bias)
    # with scale=-1, bias=-db
    db = gb2[:, 0] - gb2[:, 1]               # [3]
    cf[0:2, GB_OFF] = -db[0]
    cf[32:34, GB_OFF] = db[0]
    cf[64:66, GB_OFF] = [-db[1], db[1]]
    cf[96:98, GB_OFF] = [-db[2], db[2]]
    zero_b1 = bool(np.all(b1 == 0.0))
    return cb.astype(ml_dtypes.bfloat16), cf, zero_b1


def _bcast_src(scr, pair):
    """DRAM AP streaming rows (2p x64, 2p+1 x64) of scratch [4, TB]: matches a
    [128, TB] SBUF destination partition-major."""
    row = scr[2 * pair: 2 * pair + 1, :]
    return bass.AP(tensor=row.tensor, offset=row.offset,
                   ap=[[TB, 2], [0, 64], [1, TB]])


def _build_nc(n_tiles, zero_b1, reps=1):
    nc = bacc.Bacc("TRN2", target_bir_lowering=False)
    xt = nc.dram_tensor("xt", [D, BC], BF16, kind="ExternalInput")
    cbd = nc.dram_tensor("cb", [128, NB], BF16, kind="ExternalInput")
    cfd = nc.dram_tensor("cf", [128, NF], F32, kind="ExternalInput")
    outT = nc.dram_tensor("outT", [C, BC], F32, kind="ExternalOutput")

    AF = mybir.ActivationFunctionType
    OP = mybir.AluOpType

    with tile.TileContext(nc) as tc:
        with (
            tc.tile_pool(name="singles", bufs=1) as singles,
            tc.tile_pool(name="xp", bufs=4) as xp,
            tc.tile_pool(name="hp", bufs=2) as hp,
            tc.tile_pool(name="ep", bufs=4) as ep,
            tc.tile_pool(name="sp", bufs=8) as sp,
            tc.tile_pool(name="op", bufs=4) as op_pool,
            tc.tile_pool(name="psL1", bufs=2, space="PSUM") as psL1p,
            tc.tile_pool(name="psE", bufs=1, space="PSUM") as psEp,
            tc.tile_pool(name="psG", bufs=1, space="PSUM") as psGp,
            tc.tile_pool(name="psS", bufs=1, space="PSUM") as psSp,
            tc.tile_pool(name="psO", bufs=1, space="PSUM") as psOp,
            tc.tile_pool(name="drp", bufs=8, space="DRAM") as drp,
        ):
            cs = singles.tile([128, NB], BF16)
            nc.sync.dma_start(out=cs, in_=cbd[:, :])
            cf = singles.tile([128, NF], F32)
            nc.sync.dma_start(out=cf, in_=cfd[:, :])
            ONE = singles.tile([128, TB], F32)
            nc.vector.memset(ONE, 1.0)

            def w1_ap(u, hb):
                a = W1_OFF + u * H + hb * 128
                return cs[:, a: a + 128]

            def w2_ap(k, e):
                a = W2_OFF + (k * 4 + e) * 64
                return cs[:, a: a + 64]

            def gp_ap(k, j):
                a = GP_OFF + k * 8 + j * 2
                return cs[:, a: a + 2]

            def tile_body():
                for t in range(n_tiles):
                    one_tile(t)

            def one_tile(t):
                xtile = xp.tile([D, TB], BF16, tag="x")
                nc.sync.dma_start(out=xtile, in_=xt[:, t * TB:(t + 1) * TB])

                # ---- L1: 7 units x 4 h-blocks, paired evacuation ----
                hsb = {}
                for p in range(N_PAIRS):
                    blocks = L1_BLOCKS[2 * p: 2 * p + 2]
                    ps = psL1p.tile([128, 2 * TB], F32, tag="l1")
                    for s, (u, hb) in enumerate(blocks):
                        nc.tensor.matmul(ps[:, s * TB:(s + 1) * TB],
                                         w1_ap(u, hb), xtile,
                                         start=True, stop=True)
                    h = hp.tile([128, 2 * TB], BF16, tag=f"hp{p}", bufs=2)
                    if zero_b1:
                        if PAIR_ON_SCALAR[p]:
                            nc.scalar.activation(h, ps, AF.Relu)
                        else:
                            nc.vector.tensor_scalar(h, ps, 0.0, None,
                                                    op0=OP.max)
                    else:
                        for s, (u, hb) in enumerate(blocks):
                            bias_ap = cf[:, B1_OFF + u * KH + hb:
                                         B1_OFF + u * KH + hb + 1]
                            hs = h[:, s * TB:(s + 1) * TB]
                            pss = ps[:, s * TB:(s + 1) * TB]
                            if PAIR_ON_SCALAR[p]:
                                nc.scalar.activation(hs, pss, AF.Relu,
                                                     bias=bias_ap)
                            else:
                                nc.vector.tensor_scalar(hs, pss, bias_ap, 0.0,
                                                        op0=OP.add, op1=OP.max)
                    for s, (u, hb) in enumerate(blocks):
                        hsb[u, hb] = h[:, s * TB:(s + 1) * TB]

                # ---- L2 experts: pairs (e0,e1) and (e2,e3), col-tiled ----
                expc = ep.tile([128, 2 * TB], BF16, tag="exp")
                for pair in range(2):
                    psE = psEp.tile([128, TB], F32, tag="e2")
                    ua, ub = 3 + 2 * pair, 4 + 2 * pair
                    for k in range(KH):
                        nc.tensor.matmul(psE[0:64, :], w2_ap(k, 2 * pair),
                                         hsb[ua, k], start=(k == 0),
                                         stop=(k == KH - 1),
                                         tile_position=(0, 0))
                        nc.tensor.matmul(psE[64:128, :], w2_ap(k, 2 * pair + 1),
                                         hsb[ub, k], start=(k == 0),
                                         stop=(k == KH - 1),
                                         tile_position=(0, 64))
                    eb_ap = cf[:, EB_OFF + pair: EB_OFF + pair + 1]
                    nc.scalar.activation(expc[:, pair * TB:(pair + 1) * TB],
                                         psE, AF.Exp, bias=eb_ap)

                # ---- gates: one bank, four concurrent PE col groups ----
                psG = psGp.tile([128, TB], F32, tag="g")
                for k in range(KH):
                    st, sp_ = (k == 0), (k == KH - 1)
                    nc.tensor.matmul(psG[0:2, :], gp_ap(k, 0), hsb[0, k],
                                     start=st, stop=sp_, tile_position=(0, 0))
                    nc.tensor.matmul(psG[32:34, :], gp_ap(k, 1), hsb[0, k],
                                     start=st, stop=sp_, tile_position=(0, 32))
                    nc.tensor.matmul(psG[64:66, :], gp_ap(k, 2), hsb[1, k],
                                     start=st, stop=sp_, tile_position=(0, 64))
                    nc.tensor.matmul(psG[96:98, :], gp_ap(k, 3), hsb[2, k],
                                     start=st, stop=sp_, tile_position=(0, 96))
                # gate sigmoids via exp (Sigmoid shares no ACT table set with
                # Exp -- using it would force a 2.7us table swap every tile):
                # sigma(z) = 1/(1+e^-z); root*gate = 1/((1+Er)(1+Eg)) and the
                # per-expert softmax sum S folds into the same reciprocal.
                E = sp.tile([128, TB], F32, tag="E", bufs=4)
                nc.scalar.activation(E[0:98, :], psG[0:98, :], AF.Exp,
                                     bias=cf[0:98, GB_OFF: GB_OFF + 1],
                                     scale=-1.0)
                # replicate root rows 0:34 -> 64:98 for lane-aligned product
                rt = sp.tile([128, TB], F32, tag="rt", bufs=4)
                nc.gpsimd.dma_start(out=rt[64:98, :], in_=E[0:34, :])
                G1 = sp.tile([128, TB], F32, tag="G1", bufs=4)
                nc.gpsimd.tensor_tensor(G1[64:98, :], E[64:98, :],
                                        ONE[64:98, :], op=OP.add)
                G1b = sp.tile([128, TB], F32, tag="G1b", bufs=4)
                nc.gpsimd.tensor_tensor(G1b[64:98, :], rt[64:98, :],
                                        ONE[64:98, :], op=OP.add)
                G2 = sp.tile([128, TB], F32, tag="G2", bufs=4)
                nc.gpsimd.tensor_tensor(G2[64:98, :], G1[64:98, :],
                                        G1b[64:98, :], op=OP.mult)

                # ---- partition sums of exp via ones-select matmul ----
                psS = psSp.tile([128, TB], F32, tag="s")
                nc.tensor.matmul(psS[64:66, :], cs[:, OS_OFF: OS_OFF + 2],
                                 expc[:, 0:TB], start=True, stop=True,
                                 tile_position=(0, 64))
                nc.tensor.matmul(psS[96:98, :], cs[:, OS_OFF: OS_OFF + 2],
                                 expc[:, TB: 2 * TB], start=True, stop=True,
                                 tile_position=(0, 96))

                # ---- C = 1/((1+Er)(1+Eg)S) at rows {64,65,96,97} ----
                G3 = sp.tile([128, TB], F32, tag="G3", bufs=4)
                nc.vector.tensor_tensor(G3[64:98, :], G2[64:98, :],
                                        psS[64:98, :], op=OP.mult)
                Cf_t = sp.tile([128, TB], F32, tag="C", bufs=4)
                nc.vector.reciprocal(Cf_t[64:98, :], G3[64:98, :])

                # ---- DMA partition-broadcast of coeff rows via DRAM scratch ----
                scr = drp.tile([4, TB], F32, tag="scr")
                nc.gpsimd.dma_start(out=scr[0:2, :], in_=Cf_t[64:66, :])
                nc.gpsimd.dma_start(out=scr[2:4, :], in_=Cf_t[96:98, :])
                cbc = sp.tile([128, 2 * TB], F32, tag="cbc", bufs=4)
                nc.gpsimd.dma_start(out=cbc[:, 0:TB], in_=_bcast_src(scr, 0))
                nc.gpsimd.dma_start(out=cbc[:, TB: 2 * TB],
                                    in_=_bcast_src(scr, 1))
                prod = sp.tile([128, 2 * TB], BF16, tag="prod", bufs=4)
                nc.gpsimd.tensor_tensor(prod, expc, cbc, op=OP.mult)

                # ---- final sum of 4 experts via stacked identity ----
                psO = psOp.tile([64, TB], F32, tag="o")
                id2 = cs[:, ID_OFF: ID_OFF + 64]
                nc.tensor.matmul(psO, id2, prod[:, 0:TB], start=True, stop=False)
                nc.tensor.matmul(psO, id2, prod[:, TB: 2 * TB], start=False,
                                 stop=True)
                osb = op_pool.tile([64, TB], F32, tag="osb")
                nc.scalar.copy(osb, psO)
                nc.scalar.dma_start(out=outT[:, t * TB:(t + 1) * TB], in_=osb)

            if reps > 1:
                with tc.For_i(0, reps, 1):
                    tile_body()
            else:
                tile_body()

    nc.compile()
    return nc


def _make_looped(nc, n_cores, n_iters):
    """Jitted callable that executes the kernel NEFF n_iters times
    back-to-back on device (output buffers chained through as the next
    iteration's donor operands, so the calls serialize and can't be CSE'd).
    Used only by the local benchmark harness, not by kernel()."""
    import jax
    from jax.sharding import Mesh, PartitionSpec
    from jax.experimental.shard_map import shard_map
    from concourse import bass2jax
    import concourse.mybir as _mybir

    partition_name = (nc.partition_id_tensor.name
                      if nc.partition_id_tensor else None)
    in_names, out_names, out_avals, zero_outs = [], [], [], []
    for alloc in nc.m.functions[0].allocations:
        if not isinstance(alloc, _mybir.MemoryLocationSet):
            continue
        name = alloc.memorylocations[0].name
        if alloc.kind == "ExternalInput":
            if name != partition_name:
                in_names.append(name)
        elif alloc.kind == "ExternalOutput":
            out_names.append(name)
            shape = tuple(alloc.tensor_shape)
            dtype = _mybir.dt.np(alloc.dtype)
            out_avals.append(jax.core.ShapedArray(shape, dtype))
            zero_outs.append(np.zeros(shape, dtype))
    n_params = len(in_names)
    all_in_names = list(in_names) + list(out_names)
    if partition_name is not None:
        all_in_names.append(partition_name)

    def _body(*args):
        ins = list(args[:n_params])
        chain = list(args[n_params:])
        for _ in range(n_iters):
            operands = ins + chain
            if partition_name is not None:
                operands.append(bass2jax.partition_id_tensor())
            chain = list(bass2jax._bass_exec_p.bind(
                *operands,
                out_avals=tuple(out_avals),
                in_names=tuple(all_in_names),
                out_names=tuple(out_names),
                lowering_input_output_aliases=(),
                sim_require_finite=True,
                sim_require_nnan=True,
                nc=nc,
            ))
        return tuple(chain)

    devices = jax.devices()[:n_cores]
    mesh = Mesh(np.asarray(devices), ("core",))
    in_specs = (PartitionSpec("core"),) * (n_params + len(out_names))
    out_specs = (PartitionSpec("core"),) * len(out_names)
    fn = jax.jit(shard_map(_body, mesh=mesh, in_specs=in_specs,
                           out_specs=out_specs, check_rep=False),
                 keep_unused=True)
    return fn, in_names, out_names, zero_outs, mesh


def benchmark(inputs, n_iters=33, reps=6):
    """Measure steady-state per-execution time of the kernel across the
    8 cores: compile one NEFF running the kernel once and one running it
    n_iters times via a hardware loop; the wall-time delta divided by
    (n_iters-1) cancels the axon dispatch round-trip."""
    import time as _time
    import jax
    from jax.sharding import NamedSharding, PartitionSpec

    x = np.asarray(inputs["x"], dtype=np.float32)
    cb, cf, zero_b1 = _build_consts(
        *[np.asarray(inputs[k], np.float32) for k in
          ("gW1", "gb1", "gW2", "gb2", "eW1", "eb1", "eW2", "eb2")])
    n_rows = x.shape[0]
    bc = n_rows // NCORES
    n_tiles = bc // TB
    global BC
    BC = bc
    nc1 = _build_nc(n_tiles, zero_b1, reps=1)
    ncN = _build_nc(n_tiles, zero_b1, reps=n_iters)

    xs = x.reshape(NCORES, bc, D)
    in_maps = [
        {"xt": np.ascontiguousarray(xs[c].T).astype(ml_dtypes.bfloat16),
         "cb": cb, "cf": cf}
        for c in range(NCORES)
    ]

    fn1, in_names, out_names, zero_outs, mesh = _make_looped(nc1, NCORES, 1)
    fnN, *_ = _make_looped(ncN, NCORES, 1)

    sh = NamedSharding(mesh, PartitionSpec("core"))
    dev_args = [
        jax.device_put(
            np.concatenate([np.asarray(in_maps[c][n]) for c in range(NCORES)],
                           axis=0), sh)
        for n in in_names
    ] + [
        jax.device_put(np.concatenate([z] * NCORES, axis=0), sh)
        for z in zero_outs
    ]

    outs1 = jax.block_until_ready(fn1(*dev_args))     # compile + warm
    jax.block_until_ready(fnN(*dev_args))

    def best(fn):
        ts = []
        for _ in range(reps):
            t0 = _time.perf_counter()
            jax.block_until_ready(fn(*dev_args))
            ts.append(_time.perf_counter() - t0)
        return min(ts)

    t1, tN = best(fn1), best(fnN)
    per_exec_ns = (tN - t1) / (n_iters - 1) * 1e9

    outs = {}
    for i, name in enumerate(out_names):
        arr = np.asarray(outs1[i])
        outs[name] = arr.reshape(NCORES, -1, *arr.shape[1:])
    return per_exec_ns, outs, (t1, tN)


def kernel(x, gW1, gb1, gW2, gb2, eW1, eb1, eW2, eb2, _trace=False):
    x = np.asarray(x, dtype=np.float32)
    cb, cf, zero_b1 = _build_consts(
        np.asarray(gW1, np.float32), np.asarray(gb1, np.float32),
        np.asarray(gW2, np.float32), np.asarray(gb2, np.float32),
        np.asarray(eW1, np.float32), np.asarray(eb1, np.float32),
        np.asarray(eW2, np.float32), np.asarray(eb2, np.float32))
    n_rows = x.shape[0]
    bc = n_rows // NCORES
    n_tiles = bc // TB
    assert bc * NCORES == n_rows and n_tiles * TB == bc

    global BC
    BC = bc
    nc = _build_nc(n_tiles, zero_b1)

    xs = x.reshape(NCORES, bc, D)
    in_maps = [
        {"xt": np.ascontiguousarray(xs[c].T).astype(ml_dtypes.bfloat16),
         "cb": cb, "cf": cf}
        for c in range(NCORES)
    ]
    res = run_bass_kernel_spmd(nc, in_maps, core_ids=list(range(NCORES)),
                               trace=_trace)
    out = np.concatenate([r["outT"].T for r in res.results], axis=0)
    kernel.last_results = res
    return np.ascontiguousarray(out.astype(np.float32))
